# revision 28
# baseline (speedup 1.0000x reference)
"""Pointer-Generator Network kernel for Trainium2 (8 NeuronCores, Bass/Tile).

Shapes (hardcoded): B=16, T=512, S=2048, D=768, fp32.

Math insight: scores[b,t,s] = dec_score[b,t] + enc_score[b,s] + b_ptr, and
softmax over s is invariant to the per-row additive constant dec_score[b,t]
(and b_ptr).  The masked positions are set to the constant -1e9, whose exp
underflows to exactly 0 in fp32 in the reference as well.  Therefore

    pointer_weights[b,t,s] = exp(enc_score[b,s]) * mask[b,s] / Z[b]   (no t dep!)
    context[b,t,:]         = (sum_s pm[b,s] * enc[b,s,:]) / Z[b]      (no t dep!)
    p_gen[b,t]             = sigmoid(dec[b,t,:]@g_d + context_row[b]@g_e + b_gen)

so the heavy [B,T,S] softmax and [B,T,S]x[B,S,D] einsum collapse to an [S]-sized
softmax and a [S]x[S,D] vector-matrix product per batch, plus broadcast writes.
The kernel is then DMA/HBM-bound (write pw/ctx, read enc/dec).

Pipeline: everything is per-s-chunk (128 rows) so the DVE multiply, ACT
reduce/exp and the PE context-accumulation matmuls overlap the encoder DMA
loads; only the tiny Z-normalization is a global sync point per batch.
Partition broadcasts are done by DMA with a 0-step free-dim repeat loop.

Sharding: data-parallel over batch, 2 batches per core, no collectives.
"""

import dataclasses
import numpy as np
from contextlib import ExitStack, nullcontext

import concourse.bass as bass
import concourse.mybir as mybir
from concourse import bacc, tile
from concourse.masks import make_identity
from concourse.bass_utils import run_bass_kernel_spmd

B, T, S, D = 16, 512, 2048, 768
NCORES = 8
BPC = B // NCORES          # batches per core = 2
P = 128
NS = S // P                # 16 s-chunks
NT = T // P                # 4 t-chunks
F32 = mybir.dt.float32
I32 = mybir.dt.int32
MULT = mybir.AluOpType.mult
ADD = mybir.AluOpType.add
AXF = mybir.AxisListType.X
EXP = mybir.ActivationFunctionType.Exp
SIGMOID = mybir.ActivationFunctionType.Sigmoid
COPY = mybir.ActivationFunctionType.Copy
ND = D // P                # 6 d-slices of 128

_CACHE = {}


def _bcast(ap, n):
    """Source AP that repeats `ap`'s free contents n times (partition
    broadcast source for DMA): insert a [0, n] repeat loop after the
    partition dim."""
    a = list(ap.ap)
    return dataclasses.replace(ap, ap=[a[0], [0, n]] + a[1:])


def build_nc(loop_iters=None, bench_io=False):
    nc = bacc.Bacc("TRN2", target_bir_lowering=False)
    if bench_io:
        enc_d = nc.dram_tensor("enc", [BPC, S, D], F32)
        dec_d = nc.dram_tensor("dec", [BPC, T, D], F32)
        mask_d = nc.dram_tensor("mask", [BPC, S], I32)
        web_d = nc.dram_tensor("we_bc", [P, D], F32)
        gdb_d = nc.dram_tensor("gd_bc", [P, D], F32)
        ge_d = nc.dram_tensor("ge_row", [1, D], F32)
        bg_d = nc.dram_tensor("bgen", [1, 1], F32)
        pw_d = nc.dram_tensor("pw", [BPC, T, S], F32)
        pg_d = nc.dram_tensor("pgen", [BPC, T, 1], F32)
        cx_d = nc.dram_tensor("ctx", [BPC, T, D], F32)
        x_d = nc.declare_dram_parameter("x", [1, 16], F32, isOutput=False)
        y_d = nc.declare_dram_parameter("y", [1, 16], F32, isOutput=True)
    else:
        enc_d = nc.declare_dram_parameter("enc", [BPC, S, D], F32, isOutput=False)
        dec_d = nc.declare_dram_parameter("dec", [BPC, T, D], F32, isOutput=False)
        mask_d = nc.declare_dram_parameter("mask", [BPC, S], I32, isOutput=False)
        web_d = nc.declare_dram_parameter("we_bc", [P, D], F32, isOutput=False)
        gdb_d = nc.declare_dram_parameter("gd_bc", [P, D], F32, isOutput=False)
        ge_d = nc.declare_dram_parameter("ge_row", [1, D], F32, isOutput=False)
        bg_d = nc.declare_dram_parameter("bgen", [1, 1], F32, isOutput=False)
        pw_d = nc.declare_dram_parameter("pw", [BPC, T, S], F32, isOutput=True)
        pg_d = nc.declare_dram_parameter("pgen", [BPC, T, 1], F32, isOutput=True)
        cx_d = nc.declare_dram_parameter("ctx", [BPC, T, D], F32, isOutput=True)

    with tile.TileContext(nc) as tc, ExitStack() as ectx:
        sb = ectx.enter_context(tc.tile_pool(name="sb", bufs=1))
        ps = ectx.enter_context(tc.tile_pool(name="ps", bufs=1, space="PSUM"))

        # ---- constants / parameter loads (bufs=1 tags: live forever) ----
        identity = sb.tile([P, P], F32, tag="identity", name="identity")
        make_identity(nc, identity)
        ones_col = sb.tile([P, 1], F32, tag="ones_col", name="ones_col")
        nc.vector.memset(ones_col, 1.0)
        zbias = sb.tile([P, 1], F32, tag="zbias", name="zbias")
        nc.vector.memset(zbias, 0.0)
        we_bc = sb.tile([P, D], F32, tag="we_bc", name="we_bc")
        nc.sync.dma_start(we_bc, web_d[:, :])
        gd_bc = sb.tile([P, D], F32, tag="gd_bc", name="gd_bc")
        nc.sync.dma_start(gd_bc, gdb_d[:, :])
        ge_row = sb.tile([1, D], F32, tag="ge_row", name="ge_row")
        nc.sync.dma_start(ge_row, ge_d[:, :])
        bgen_sb = sb.tile([1, 1], F32, tag="bgen", name="bgen_sb")
        nc.sync.dma_start(bgen_sb, bg_d[:, :])
        if bench_io:
            xt = sb.tile([1, 16], F32, tag="xt", name="xt")
            nc.sync.dma_start(xt, x_d[:, :])

        env = dict(enc_d=enc_d, dec_d=dec_d, mask_d=mask_d, pw_d=pw_d,
                   pg_d=pg_d, cx_d=cx_d, identity=identity, ones_col=ones_col,
                   zbias=zbias, we_bc=we_bc, gd_bc=gd_bc, ge_row=ge_row,
                   bgen_sb=bgen_sb)
        loop_cm = tc.For_i(0, loop_iters, 1) if loop_iters else nullcontext()
        with loop_cm:
            body(nc, tc, sb, ps, env)
        if bench_io:
            nc.sync.dma_start(y_d[:, :], xt)

    nc.compile()
    return nc


def body(nc, tc, sb, ps, env):
    enc_d, dec_d, mask_d = env["enc_d"], env["dec_d"], env["mask_d"]
    pw_d, pg_d, cx_d = env["pw_d"], env["pg_d"], env["cx_d"]
    identity, ones_col, zbias = env["identity"], env["ones_col"], env["zbias"]
    we_bc, gd_bc, ge_row, bgen_sb = (env["we_bc"], env["gd_bc"],
                                     env["ge_row"], env["bgen_sb"])

    for b in range(BPC):
        # ---- mask first: load [16,128] i32, cast f32, PE-transpose ----
        mask_i = sb.tile([NS, P], I32, tag="mask_i", bufs=2, name=f"mask_i_{b}")
        nc.sync.dma_start(mask_i, mask_d[b].rearrange("(i f) -> i f", i=NS))
        mask_f = sb.tile([NS, P], F32, tag="mask_f", bufs=2, name=f"mask_f_{b}")
        nc.vector.tensor_copy(mask_f, mask_i)
        maskT_ps = ps.tile([P, NS], F32, tag="tp", bufs=2, name=f"maskT_ps_{b}")
        nc.tensor.transpose(maskT_ps, mask_f, identity[:NS, :NS])

        scores = sb.tile([P, NS], F32, tag="scores", bufs=2, name=f"scores_{b}")
        exp_c = sb.tile([P, NS], F32, tag="exp_c", bufs=2, name=f"exp_c_{b}")
        pm = sb.tile([P, NS], F32, tag="pm", bufs=2, name=f"pm_{b}")
        # context accumulator: column j holds ctx[128*j : 128*(j+1)] on the
        # partition dim.  enc slices go in as the (row-parallel) stationary
        # operand, the pm column streams as a single fp32 moving column.
        ctx6_ps = ps.tile([P, ND], F32, tag="ctx", bufs=2, name=f"ctx6_ps_{b}")
        # start=True on any matmul zeroes the whole PSUM bank, which would
        # wipe sibling columns' already-accumulated chunk-0 results -- so
        # zero explicitly and accumulate with start=False throughout.
        nc.vector.memset(ctx6_ps, 0.0)

        # ---- per-chunk pipeline: load -> mult -> reduce -> exp -> mask ->
        #      PE context accumulation.  Everything overlaps the DMA loads.
        enc_t = []
        for i in range(NS):
            et = sb.tile([P, D], F32, tag="enc", bufs=2 * NS, name=f"enc_{b}_{i}")
            nc.sync.dma_start(et, enc_d[b, P * i:P * (i + 1), :])
            enc_t.append(et)
            jt = sb.tile([P, D], F32, tag="junk", bufs=5, name=f"junk_{b}_{i}")
            nc.vector.tensor_tensor(out=jt, in0=et, in1=we_bc, op=MULT)
            jt2 = sb.tile([P, D], F32, tag="junk", bufs=5, name=f"junk2_{b}_{i}")
            nc.scalar.activation(jt2, jt, COPY, accum_out=scores[:, i:i + 1])
            nc.scalar.activation(exp_c[:, i:i + 1], scores[:, i:i + 1], EXP,
                                 bias=zbias)
            nc.vector.tensor_tensor(out=pm[:, i:i + 1], in0=exp_c[:, i:i + 1],
                                    in1=maskT_ps[:, i:i + 1], op=MULT)
            for j in range(ND):
                nc.tensor.matmul(ctx6_ps[:, j:j + 1],
                                 et[:, P * j:P * (j + 1)], pm[:, i:i + 1],
                                 start=False, stop=(i == NS - 1),
                                 skip_group_check=True)

        # ---- Z = sum(pm); rinv = 1/Z; broadcast via DMA ----
        rowsum = sb.tile([P, 1], F32, tag="rowsum", bufs=2, name=f"rowsum_{b}")
        nc.vector.tensor_reduce(out=rowsum, in_=pm, axis=AXF, op=ADD)
        z_ps = ps.tile([1, 1], F32, tag="zz", bufs=2, name=f"z_ps_{b}")
        nc.tensor.matmul(z_ps, rowsum, ones_col, start=True, stop=True)
        rinv = sb.tile([1, 1], F32, tag="rinv", bufs=2, name=f"rinv_{b}")
        nc.vector.reciprocal(rinv, z_ps)
        rinv_bc = sb.tile([P, 1], F32, tag="rinv_bc", bufs=2, name=f"rinv_bc_{b}")
        nc.sync.dma_start(rinv_bc, _bcast(rinv[0:1, 0:1], P))

        # ---- normalized p row: pm_n -> PE transpose -> [1, 2048] row ----
        pm_n = sb.tile([P, NS], F32, tag="pm_n", bufs=2, name=f"pm_n_{b}")
        nc.vector.tensor_scalar_mul(pm_n, pm, rinv_bc)
        pmT_ps = ps.tile([NS, P], F32, tag="tp", bufs=2, name=f"pmT_ps_{b}")
        nc.tensor.transpose(pmT_ps, pm_n, identity)
        pmT = sb.tile([NS, P], F32, tag="pmT", bufs=2, name=f"pmT_{b}")
        nc.scalar.copy(pmT, pmT_ps)
        p_row = sb.tile([1, S], F32, tag="p_row", bufs=2, name=f"p_row_{b}")
        nc.sync.dma_start(p_row.rearrange("a (i f) -> a i f", i=NS), pmT)

        # ---- pointer_weights: DMA-broadcast p_row to 128 partitions, out ----
        pw_t = sb.tile([P, S], F32, tag="pw_t", bufs=2, name=f"pw_t_{b}")
        nc.sync.dma_start(pw_t, _bcast(p_row[0:1, :], P))
        for k in range(NT):
            nc.sync.dma_start(pw_d[b, P * k:P * (k + 1), :], pw_t)

        # ---- context row: normalize, transpose to a row, broadcast, out ----
        ctxn = sb.tile([P, ND], F32, tag="ctxn", bufs=2, name=f"ctxn_{b}")
        nc.vector.tensor_scalar_mul(ctxn, ctx6_ps, rinv_bc)
        ctxT_ps = ps.tile([ND, P], F32, tag="tp", bufs=2, name=f"ctxT_ps_{b}")
        nc.tensor.transpose(ctxT_ps, ctxn, identity)
        ctxT = sb.tile([ND, P], F32, tag="ctxT", bufs=2, name=f"ctxT_{b}")
        nc.scalar.copy(ctxT, ctxT_ps)
        ctx_row = sb.tile([1, D], F32, tag="ctx_row", bufs=2, name=f"ctx_row_{b}")
        nc.sync.dma_start(ctx_row.rearrange("a (j f) -> a j f", j=ND), ctxT)
        ctx_t = sb.tile([P, D], F32, tag="ctx_t", bufs=2, name=f"ctx_t_{b}")
        nc.sync.dma_start(ctx_t, _bcast(ctx_row[0:1, :], P))
        for k in range(NT):
            nc.sync.dma_start(cx_d[b, P * k:P * (k + 1), :], ctx_t)

        # ---- p_gen ----
        dsc = sb.tile([P, NT], F32, tag="dsc", bufs=2, name=f"dsc_{b}")
        for k in range(NT):
            dt_ = sb.tile([P, D], F32, tag="dec", bufs=2 * NT, name=f"dec_{b}_{k}")
            nc.sync.dma_start(dt_, dec_d[b, P * k:P * (k + 1), :])
            jd = sb.tile([P, D], F32, tag="junk", bufs=5, name=f"junkd_{b}_{k}")
            nc.vector.tensor_tensor(out=jd, in0=dt_, in1=gd_bc, op=MULT)
            jd2 = sb.tile([P, D], F32, tag="junk", bufs=5, name=f"junkd2_{b}_{k}")
            nc.scalar.activation(jd2, jd, COPY, accum_out=dsc[:, k:k + 1])
        # c2 = ctx_row . g_e + b_gen, broadcast to [128, 1] by DMA
        jr = sb.tile([1, D], F32, tag="junk1", bufs=2, name=f"jr_{b}")
        jr2 = sb.tile([1, D], F32, tag="junk1", bufs=2, name=f"jr2_{b}")
        c2 = sb.tile([1, 1], F32, tag="c2", bufs=2, name=f"c2_{b}")
        nc.vector.tensor_tensor(out=jr, in0=ctx_row, in1=ge_row, op=MULT)
        nc.scalar.activation(jr2, jr, COPY, accum_out=c2)
        c2b = sb.tile([1, 1], F32, tag="c2b", bufs=2, name=f"c2b_{b}")
        nc.vector.tensor_add(c2b, c2, bgen_sb)
        gb = sb.tile([P, 1], F32, tag="gb", bufs=2, name=f"gb_{b}")
        nc.sync.dma_start(gb, _bcast(c2b[0:1, 0:1], P))
        pgen_t = sb.tile([P, NT], F32, tag="pgen_t", bufs=2, name=f"pgen_t_{b}")
        nc.scalar.activation(pgen_t, dsc, SIGMOID, bias=gb)
        pgT_ps = ps.tile([NT, P], F32, tag="tp", bufs=2, name=f"pgT_ps_{b}")
        nc.tensor.transpose(pgT_ps, pgen_t, identity)
        pgT = sb.tile([NT, P], F32, tag="pgT", bufs=2, name=f"pgT_{b}")
        nc.scalar.copy(pgT, pgT_ps)
        nc.sync.dma_start(pg_d[b].rearrange("(k f) a -> k (f a)", k=NT), pgT)


def _run(inputs, trace=False, trace_cores=None, loop_iters=None):
    key = ("nc", loop_iters)
    if key not in _CACHE:
        _CACHE[key] = build_nc(loop_iters)
    nc = _CACHE[key]

    dec = np.ascontiguousarray(np.asarray(inputs["decoder_hidden"], dtype=np.float32))
    enc = np.ascontiguousarray(np.asarray(inputs["encoder_outputs"], dtype=np.float32))
    mask = np.ascontiguousarray(np.asarray(inputs["encoder_mask"], dtype=np.int32))
    w_ptr = np.asarray(inputs["w_ptr"], dtype=np.float32)
    w_gen = np.asarray(inputs["w_gen"], dtype=np.float32)
    b_gen = np.asarray(inputs["b_gen"], dtype=np.float32)
    we_bc = np.ascontiguousarray(np.broadcast_to(w_ptr[D:], (P, D)))
    gd_bc = np.ascontiguousarray(np.broadcast_to(w_gen[:D], (P, D)))
    ge_row = np.ascontiguousarray(w_gen[D:].reshape(1, D))
    bgen = np.ascontiguousarray(b_gen.reshape(1, 1))

    in_maps = []
    for c in range(NCORES):
        lo, hi = BPC * c, BPC * (c + 1)
        in_maps.append({
            "enc": np.ascontiguousarray(enc[lo:hi]),
            "dec": np.ascontiguousarray(dec[lo:hi]),
            "mask": np.ascontiguousarray(mask[lo:hi]),
            "we_bc": we_bc, "gd_bc": gd_bc, "ge_row": ge_row, "bgen": bgen,
        })

    res = run_bass_kernel_spmd(nc, in_maps, list(range(NCORES)),
                               trace=trace, trace_cores=trace_cores)
    pw = np.concatenate([r["pw"] for r in res.results], axis=0)
    pgen = np.concatenate([r["pgen"] for r in res.results], axis=0)
    ctx = np.concatenate([r["ctx"] for r in res.results], axis=0)
    return (pw, pgen, ctx), res


def kernel(decoder_hidden, encoder_outputs, encoder_mask, w_ptr, b_ptr,
           w_gen, b_gen):
    outs, _ = _run({
        "decoder_hidden": decoder_hidden,
        "encoder_outputs": encoder_outputs,
        "encoder_mask": encoder_mask,
        "w_ptr": w_ptr, "b_ptr": b_ptr, "w_gen": w_gen, "b_gen": b_gen,
    })
    return outs


# revision 35
# speedup vs baseline: 1.5425x; 1.5425x over previous
"""Pointer-Generator Network kernel for Trainium2 (8 NeuronCores, Bass/Tile).

Shapes (hardcoded): B=16, T=512, S=2048, D=768, fp32.

Math insight: scores[b,t,s] = dec_score[b,t] + enc_score[b,s] + b_ptr, and
softmax over s is invariant to the per-row additive constant dec_score[b,t]
(and b_ptr).  The masked positions are set to the constant -1e9, whose exp
underflows to exactly 0 in fp32 in the reference as well.  Therefore

    pointer_weights[b,t,s] = exp(enc_score[b,s]) * mask[b,s] / Z[b]   (no t dep!)
    context[b,t,:]         = (sum_s pm[b,s] * enc[b,s,:]) / Z[b]      (no t dep!)
    p_gen[b,t]             = sigmoid(dec[b,t,:]@g_d + context_row[b]@g_e + b_gen)

so the heavy [B,T,S] softmax and [B,T,S]x[B,S,D] einsum collapse to an [S]-sized
softmax and a [S]x[S,D] vector-matrix product per batch, plus broadcast writes.
The kernel is then DMA/HBM-bound (write pw/ctx, read enc/dec).

Pipeline: everything is per-s-chunk (128 rows) so the DVE multiply, ACT
reduce/exp and the PE context-accumulation matmuls overlap the encoder DMA
loads; only the tiny Z-normalization is a global sync point per batch.
Partition broadcasts are done by DMA with a 0-step free-dim repeat loop.

Sharding: data-parallel over batch, 2 batches per core, no collectives.
"""

import dataclasses
import numpy as np
from contextlib import ExitStack, nullcontext

import concourse.bass as bass
import concourse.mybir as mybir
from concourse import bacc, tile
from concourse.masks import make_identity
from concourse.bass_utils import run_bass_kernel_spmd

B, T, S, D = 16, 512, 2048, 768
NCORES = 8
BPC = B // NCORES          # batches per core = 2
P = 128
NS = S // P                # 16 s-chunks
NT = T // P                # 4 t-chunks
F32 = mybir.dt.float32
I32 = mybir.dt.int32
MULT = mybir.AluOpType.mult
ADD = mybir.AluOpType.add
AXF = mybir.AxisListType.X
EXP = mybir.ActivationFunctionType.Exp
SIGMOID = mybir.ActivationFunctionType.Sigmoid
COPY = mybir.ActivationFunctionType.Copy
ND = D // P                # 6 d-slices of 128

_CACHE = {}


def _bcast(ap, n):
    """Source AP that repeats `ap`'s free contents n times (partition
    broadcast source for DMA): insert a [0, n] repeat loop after the
    partition dim."""
    a = list(ap.ap)
    return dataclasses.replace(ap, ap=[a[0], [0, n]] + a[1:])


def build_nc(loop_iters=None, bench_io=False):
    nc = bacc.Bacc("TRN2", target_bir_lowering=False)
    if bench_io:
        enc_d = nc.dram_tensor("enc", [BPC, S, D], F32)
        dec_d = nc.dram_tensor("dec", [BPC, T, D], F32)
        mask_d = nc.dram_tensor("mask", [BPC, S], I32)
        web_d = nc.dram_tensor("we_bc", [P, D], F32)
        gdb_d = nc.dram_tensor("gd_bc", [P, D], F32)
        ge_d = nc.dram_tensor("ge_row", [1, D], F32)
        bg_d = nc.dram_tensor("bgen", [1, 1], F32)
        pw_d = nc.dram_tensor("pw", [BPC, T, S], F32)
        pg_d = nc.dram_tensor("pgen", [BPC, T, 1], F32)
        cx_d = nc.dram_tensor("ctx", [BPC, T, D], F32)
        x_d = nc.declare_dram_parameter("x", [1, 16], F32, isOutput=False)
        y_d = nc.declare_dram_parameter("y", [1, 16], F32, isOutput=True)
    else:
        enc_d = nc.declare_dram_parameter("enc", [BPC, S, D], F32, isOutput=False)
        dec_d = nc.declare_dram_parameter("dec", [BPC, T, D], F32, isOutput=False)
        mask_d = nc.declare_dram_parameter("mask", [BPC, S], I32, isOutput=False)
        web_d = nc.declare_dram_parameter("we_bc", [P, D], F32, isOutput=False)
        gdb_d = nc.declare_dram_parameter("gd_bc", [P, D], F32, isOutput=False)
        ge_d = nc.declare_dram_parameter("ge_row", [1, D], F32, isOutput=False)
        bg_d = nc.declare_dram_parameter("bgen", [1, 1], F32, isOutput=False)
        pw_d = nc.declare_dram_parameter("pw", [BPC, T, S], F32, isOutput=True)
        pg_d = nc.declare_dram_parameter("pgen", [BPC, T, 1], F32, isOutput=True)
        cx_d = nc.declare_dram_parameter("ctx", [BPC, T, D], F32, isOutput=True)

    with tile.TileContext(nc) as tc, ExitStack() as ectx:
        sb = ectx.enter_context(tc.tile_pool(name="sb", bufs=1))
        ps = ectx.enter_context(tc.tile_pool(name="ps", bufs=1, space="PSUM"))

        # ---- constants / parameter loads (bufs=1 tags: live forever) ----
        identity = sb.tile([P, P], F32, tag="identity", name="identity")
        make_identity(nc, identity)
        ones_col = sb.tile([P, 1], F32, tag="ones_col", name="ones_col")
        nc.vector.memset(ones_col, 1.0)
        ones_row = sb.tile([1, P], F32, tag="ones_row", name="ones_row")
        nc.vector.memset(ones_row, 1.0)
        zbias = sb.tile([P, 1], F32, tag="zbias", name="zbias")
        nc.vector.memset(zbias, 0.0)
        we_bc = sb.tile([P, D], F32, tag="we_bc", name="we_bc")
        nc.sync.dma_start(we_bc, web_d[:, :])
        gd_bc = sb.tile([P, D], F32, tag="gd_bc", name="gd_bc")
        nc.sync.dma_start(gd_bc, gdb_d[:, :])
        ge_row = sb.tile([1, D], F32, tag="ge_row", name="ge_row")
        nc.sync.dma_start(ge_row, ge_d[:, :])
        bgen_sb = sb.tile([1, 1], F32, tag="bgen", name="bgen_sb")
        nc.sync.dma_start(bgen_sb, bg_d[:, :])
        if bench_io:
            xt = sb.tile([1, 16], F32, tag="xt", name="xt")
            nc.sync.dma_start(xt, x_d[:, :])

        env = dict(enc_d=enc_d, dec_d=dec_d, mask_d=mask_d, pw_d=pw_d,
                   pg_d=pg_d, cx_d=cx_d, identity=identity, ones_col=ones_col,
                   ones_row=ones_row, zbias=zbias, we_bc=we_bc, gd_bc=gd_bc,
                   ge_row=ge_row, bgen_sb=bgen_sb)
        loop_cm = tc.For_i(0, loop_iters, 1) if loop_iters else nullcontext()
        with loop_cm:
            body(nc, tc, sb, ps, env)
        if bench_io:
            nc.sync.dma_start(y_d[:, :], xt)

    nc.compile()
    return nc


def body(nc, tc, sb, ps, env):
    enc_d, dec_d, mask_d = env["enc_d"], env["dec_d"], env["mask_d"]
    pw_d, pg_d, cx_d = env["pw_d"], env["pg_d"], env["cx_d"]
    identity, ones_col, zbias = env["identity"], env["ones_col"], env["zbias"]
    ones_row = env["ones_row"]
    we_bc, gd_bc, ge_row, bgen_sb = (env["we_bc"], env["gd_bc"],
                                     env["ge_row"], env["bgen_sb"])

    for b in range(BPC):
        # ---- mask first: load [16,128] i32, cast f32, PE-transpose ----
        mask_i = sb.tile([NS, P], I32, tag="mask_i", bufs=2, name=f"mask_i_{b}")
        nc.sync.dma_start(mask_i, mask_d[b].rearrange("(i f) -> i f", i=NS))
        mask_f = sb.tile([NS, P], F32, tag="mask_f", bufs=2, name=f"mask_f_{b}")
        nc.vector.tensor_copy(mask_f, mask_i)
        maskT_ps = ps.tile([P, NS], F32, tag="tp", bufs=2, name=f"maskT_ps_{b}")
        nc.tensor.transpose(maskT_ps, mask_f, identity[:NS, :NS])

        scores = sb.tile([P, NS], F32, tag="scores", bufs=2, name=f"scores_{b}")
        exp_c = sb.tile([P, NS], F32, tag="exp_c", bufs=2, name=f"exp_c_{b}")
        pm = sb.tile([P, NS], F32, tag="pm", bufs=2, name=f"pm_{b}")
        # context accumulator: column j holds ctx[128*j : 128*(j+1)] on the
        # partition dim.  enc slices go in as the (row-parallel) stationary
        # operand, the pm column streams as a single fp32 moving column.
        ctx6_ps = ps.tile([P, ND], F32, tag="ctx", bufs=2, name=f"ctx6_ps_{b}")
        # start=True on any matmul zeroes the whole PSUM bank, which would
        # wipe sibling columns' already-accumulated chunk-0 results -- so
        # zero explicitly and accumulate with start=False throughout.
        nc.vector.memset(ctx6_ps, 0.0)

        # ---- per-chunk pipeline: load -> mult -> reduce -> exp -> mask ->
        #      PE context accumulation.  Everything overlaps the DMA loads.
        enc_t = []
        for i in range(NS):
            et = sb.tile([P, D], F32, tag="enc", bufs=2 * NS, name=f"enc_{b}_{i}")
            nc.sync.dma_start(et, enc_d[b, P * i:P * (i + 1), :])
            enc_t.append(et)
            jt = sb.tile([P, D], F32, tag="junk", bufs=5, name=f"junk_{b}_{i}")
            nc.vector.tensor_tensor(out=jt, in0=et, in1=we_bc, op=MULT)
            jt2 = sb.tile([P, D], F32, tag="junk", bufs=5, name=f"junk2_{b}_{i}")
            nc.scalar.activation(jt2, jt, COPY, accum_out=scores[:, i:i + 1])
            nc.scalar.activation(exp_c[:, i:i + 1], scores[:, i:i + 1], EXP,
                                 bias=zbias)
            nc.vector.tensor_tensor(out=pm[:, i:i + 1], in0=exp_c[:, i:i + 1],
                                    in1=maskT_ps[:, i:i + 1], op=MULT)
            for j in range(ND):
                nc.tensor.matmul(ctx6_ps[:, j:j + 1],
                                 et[:, P * j:P * (j + 1)], pm[:, i:i + 1],
                                 start=False, stop=(i == NS - 1),
                                 skip_group_check=True)

        # ---- Z = sum(pm); rinv = 1/Z; broadcast via DMA ----
        rowsum = sb.tile([P, 1], F32, tag="rowsum", bufs=2, name=f"rowsum_{b}")
        nc.vector.tensor_reduce(out=rowsum, in_=pm, axis=AXF, op=ADD)
        z_ps = ps.tile([1, 1], F32, tag="zz", bufs=2, name=f"z_ps_{b}")
        nc.tensor.matmul(z_ps, rowsum, ones_col, start=True, stop=True)
        rinv = sb.tile([1, 1], F32, tag="rinv", bufs=2, name=f"rinv_{b}")
        nc.vector.reciprocal(rinv, z_ps)
        rb_ps = ps.tile([P, 1], F32, tag="zz", bufs=2, name=f"rb_ps_{b}")
        nc.tensor.matmul(rb_ps, ones_row, rinv, start=True, stop=True)
        rinv_bc = sb.tile([P, 1], F32, tag="rinv_bc", bufs=2, name=f"rinv_bc_{b}")
        nc.scalar.copy(rinv_bc, rb_ps)

        # ---- normalized p row: pm_n -> PE transpose -> [1, 2048] row ----
        pm_n = sb.tile([P, NS], F32, tag="pm_n", bufs=2, name=f"pm_n_{b}")
        nc.vector.tensor_scalar_mul(pm_n, pm, rinv_bc)
        pmT_ps = ps.tile([NS, P], F32, tag="tp", bufs=2, name=f"pmT_ps_{b}")
        nc.tensor.transpose(pmT_ps, pm_n, identity)
        pmT = sb.tile([NS, P], F32, tag="pmT", bufs=2, name=f"pmT_{b}")
        nc.scalar.copy(pmT, pmT_ps)
        p_row = sb.tile([1, S], F32, tag="p_row", bufs=2, name=f"p_row_{b}")
        nc.sync.dma_start(p_row.rearrange("a (i f) -> a i f", i=NS), pmT)

        # ---- pointer_weights: PE ones-matmul broadcast (reads the row once,
        #      fans out through the array), DVE copy to SBUF, out ----
        pw_t = sb.tile([P, S], F32, tag="pw_t", bufs=2, name=f"pw_t_{b}")
        for j in range(S // 512):
            bc_ps = ps.tile([P, 512], F32, tag="pwb", bufs=2,
                            name=f"pw_bc_{b}_{j}")
            nc.tensor.matmul(bc_ps, ones_row, p_row[0:1, 512 * j:512 * (j + 1)],
                             start=True, stop=True)
            nc.vector.tensor_copy(pw_t[:, 512 * j:512 * (j + 1)], bc_ps)
        for k in range(NT):
            nc.sync.dma_start(pw_d[b, P * k:P * (k + 1), :], pw_t)

        # ---- context row: normalize, transpose to a row, broadcast, out ----
        ctxn = sb.tile([P, ND], F32, tag="ctxn", bufs=2, name=f"ctxn_{b}")
        nc.vector.tensor_scalar_mul(ctxn, ctx6_ps, rinv_bc)
        ctxT_ps = ps.tile([ND, P], F32, tag="tp", bufs=2, name=f"ctxT_ps_{b}")
        nc.tensor.transpose(ctxT_ps, ctxn, identity)
        ctxT = sb.tile([ND, P], F32, tag="ctxT", bufs=2, name=f"ctxT_{b}")
        nc.scalar.copy(ctxT, ctxT_ps)
        ctx_row = sb.tile([1, D], F32, tag="ctx_row", bufs=2, name=f"ctx_row_{b}")
        nc.sync.dma_start(ctx_row.rearrange("a (j f) -> a j f", j=ND), ctxT)
        ctx_t = sb.tile([P, D], F32, tag="ctx_t", bufs=2, name=f"ctx_t_{b}")
        for j, (c0, cn) in enumerate([(0, 512), (512, 256)]):
            cb_ps = ps.tile([P, 512], F32, tag="pwb", bufs=2,
                            name=f"cb_ps_{b}_{j}")
            nc.tensor.matmul(cb_ps[:, :cn], ones_row, ctx_row[0:1, c0:c0 + cn],
                             start=True, stop=True)
            nc.vector.tensor_copy(ctx_t[:, c0:c0 + cn], cb_ps[:, :cn])
        for k in range(NT):
            nc.sync.dma_start(cx_d[b, P * k:P * (k + 1), :], ctx_t)

        # ---- p_gen ----
        dsc = sb.tile([P, NT], F32, tag="dsc", bufs=2, name=f"dsc_{b}")
        for k in range(NT):
            dt_ = sb.tile([P, D], F32, tag="dec", bufs=2 * NT, name=f"dec_{b}_{k}")
            nc.sync.dma_start(dt_, dec_d[b, P * k:P * (k + 1), :])
            jd = sb.tile([P, D], F32, tag="junk", bufs=5, name=f"junkd_{b}_{k}")
            nc.vector.tensor_tensor(out=jd, in0=dt_, in1=gd_bc, op=MULT)
            jd2 = sb.tile([P, D], F32, tag="junk", bufs=5, name=f"junkd2_{b}_{k}")
            nc.scalar.activation(jd2, jd, COPY, accum_out=dsc[:, k:k + 1])
        # c2 = ctx_row . g_e + b_gen, broadcast to [128, 1] by DMA
        jr = sb.tile([1, D], F32, tag="junk1", bufs=2, name=f"jr_{b}")
        jr2 = sb.tile([1, D], F32, tag="junk1", bufs=2, name=f"jr2_{b}")
        c2 = sb.tile([1, 1], F32, tag="c2", bufs=2, name=f"c2_{b}")
        nc.vector.tensor_tensor(out=jr, in0=ctx_row, in1=ge_row, op=MULT)
        nc.scalar.activation(jr2, jr, COPY, accum_out=c2)
        c2b = sb.tile([1, 1], F32, tag="c2b", bufs=2, name=f"c2b_{b}")
        nc.vector.tensor_add(c2b, c2, bgen_sb)
        gb_ps = ps.tile([P, 1], F32, tag="zz", bufs=2, name=f"gb_ps_{b}")
        nc.tensor.matmul(gb_ps, ones_row, c2b, start=True, stop=True)
        gb = sb.tile([P, 1], F32, tag="gb", bufs=2, name=f"gb_{b}")
        nc.scalar.copy(gb, gb_ps)
        pgen_t = sb.tile([P, NT], F32, tag="pgen_t", bufs=2, name=f"pgen_t_{b}")
        nc.scalar.activation(pgen_t, dsc, SIGMOID, bias=gb)
        pgT_ps = ps.tile([NT, P], F32, tag="tp", bufs=2, name=f"pgT_ps_{b}")
        nc.tensor.transpose(pgT_ps, pgen_t, identity)
        pgT = sb.tile([NT, P], F32, tag="pgT", bufs=2, name=f"pgT_{b}")
        nc.scalar.copy(pgT, pgT_ps)
        nc.sync.dma_start(pg_d[b].rearrange("(k f) a -> k (f a)", k=NT), pgT)


def _run(inputs, trace=False, trace_cores=None, loop_iters=None):
    key = ("nc", loop_iters)
    if key not in _CACHE:
        _CACHE[key] = build_nc(loop_iters)
    nc = _CACHE[key]

    dec = np.ascontiguousarray(np.asarray(inputs["decoder_hidden"], dtype=np.float32))
    enc = np.ascontiguousarray(np.asarray(inputs["encoder_outputs"], dtype=np.float32))
    mask = np.ascontiguousarray(np.asarray(inputs["encoder_mask"], dtype=np.int32))
    w_ptr = np.asarray(inputs["w_ptr"], dtype=np.float32)
    w_gen = np.asarray(inputs["w_gen"], dtype=np.float32)
    b_gen = np.asarray(inputs["b_gen"], dtype=np.float32)
    we_bc = np.ascontiguousarray(np.broadcast_to(w_ptr[D:], (P, D)))
    gd_bc = np.ascontiguousarray(np.broadcast_to(w_gen[:D], (P, D)))
    ge_row = np.ascontiguousarray(w_gen[D:].reshape(1, D))
    bgen = np.ascontiguousarray(b_gen.reshape(1, 1))

    in_maps = []
    for c in range(NCORES):
        lo, hi = BPC * c, BPC * (c + 1)
        in_maps.append({
            "enc": np.ascontiguousarray(enc[lo:hi]),
            "dec": np.ascontiguousarray(dec[lo:hi]),
            "mask": np.ascontiguousarray(mask[lo:hi]),
            "we_bc": we_bc, "gd_bc": gd_bc, "ge_row": ge_row, "bgen": bgen,
        })

    res = run_bass_kernel_spmd(nc, in_maps, list(range(NCORES)),
                               trace=trace, trace_cores=trace_cores)
    pw = np.concatenate([r["pw"] for r in res.results], axis=0)
    pgen = np.concatenate([r["pgen"] for r in res.results], axis=0)
    ctx = np.concatenate([r["ctx"] for r in res.results], axis=0)
    return (pw, pgen, ctx), res


def kernel(decoder_hidden, encoder_outputs, encoder_mask, w_ptr, b_ptr,
           w_gen, b_gen):
    outs, _ = _run({
        "decoder_hidden": decoder_hidden,
        "encoder_outputs": encoder_outputs,
        "encoder_mask": encoder_mask,
        "w_ptr": w_ptr, "b_ptr": b_ptr, "w_gen": w_gen, "b_gen": b_gen,
    })
    return outs


# revision 41
# speedup vs baseline: 1.7284x; 1.1205x over previous
"""Pointer-Generator Network kernel for Trainium2 (8 NeuronCores, Bass/Tile).

Shapes (hardcoded): B=16, T=512, S=2048, D=768, fp32.

Math insight: scores[b,t,s] = dec_score[b,t] + enc_score[b,s] + b_ptr, and
softmax over s is invariant to the per-row additive constant dec_score[b,t]
(and b_ptr).  The masked positions are set to the constant -1e9, whose exp
underflows to exactly 0 in fp32 in the reference as well.  Therefore

    pointer_weights[b,t,s] = exp(enc_score[b,s]) * mask[b,s] / Z[b]   (no t dep!)
    context[b,t,:]         = (sum_s pm[b,s] * enc[b,s,:]) / Z[b]      (no t dep!)
    p_gen[b,t]             = sigmoid(dec[b,t,:]@g_d + context_row[b]@g_e + b_gen)

so the heavy [B,T,S] softmax and [B,T,S]x[B,S,D] einsum collapse to an [S]-sized
softmax and a [S]x[S,D] vector-matrix product per batch, plus broadcast writes.
The kernel is then DMA/HBM-bound (write pw/ctx, read enc/dec).

Pipeline: everything is per-s-chunk (128 rows) so the DVE multiply, ACT
reduce/exp and the PE context-accumulation matmuls overlap the encoder DMA
loads; only the tiny Z-normalization is a global sync point per batch.
Partition broadcasts are done by DMA with a 0-step free-dim repeat loop.

Sharding: data-parallel over batch, 2 batches per core, no collectives.
"""

import dataclasses
import numpy as np
from contextlib import ExitStack, nullcontext

import concourse.bass as bass
import concourse.mybir as mybir
from concourse import bacc, tile
from concourse.masks import make_identity
from concourse.bass_utils import run_bass_kernel_spmd

B, T, S, D = 16, 512, 2048, 768
NCORES = 8
BPC = B // NCORES          # batches per core = 2
P = 128
NS = S // P                # 16 s-chunks
NT = T // P                # 4 t-chunks
F32 = mybir.dt.float32
I32 = mybir.dt.int32
MULT = mybir.AluOpType.mult
ADD = mybir.AluOpType.add
AXF = mybir.AxisListType.X
EXP = mybir.ActivationFunctionType.Exp
SIGMOID = mybir.ActivationFunctionType.Sigmoid
COPY = mybir.ActivationFunctionType.Copy
ND = D // P                # 6 d-slices of 128
CTX_MODE = "streaming"    # "stationary" (enc as weights) | "streaming"

_CACHE = {}


def _bcast(ap, n):
    """Source AP that repeats `ap`'s free contents n times (partition
    broadcast source for DMA): insert a [0, n] repeat loop after the
    partition dim."""
    a = list(ap.ap)
    return dataclasses.replace(ap, ap=[a[0], [0, n]] + a[1:])


def build_nc(loop_iters=None, bench_io=False):
    nc = bacc.Bacc("TRN2", target_bir_lowering=False)
    if bench_io:
        enc_d = nc.dram_tensor("enc", [BPC, S, D], F32)
        dec_d = nc.dram_tensor("dec", [BPC, T, D], F32)
        mask_d = nc.dram_tensor("mask", [BPC, S], I32)
        web_d = nc.dram_tensor("we_bc", [P, D], F32)
        gdb_d = nc.dram_tensor("gd_bc", [P, D], F32)
        ge_d = nc.dram_tensor("ge_row", [1, D], F32)
        bg_d = nc.dram_tensor("bgen", [1, 1], F32)
        pw_d = nc.dram_tensor("pw", [BPC, T, S], F32)
        pg_d = nc.dram_tensor("pgen", [BPC, T, 1], F32)
        cx_d = nc.dram_tensor("ctx", [BPC, T, D], F32)
        x_d = nc.declare_dram_parameter("x", [1, 16], F32, isOutput=False)
        y_d = nc.declare_dram_parameter("y", [1, 16], F32, isOutput=True)
    else:
        enc_d = nc.declare_dram_parameter("enc", [BPC, S, D], F32, isOutput=False)
        dec_d = nc.declare_dram_parameter("dec", [BPC, T, D], F32, isOutput=False)
        mask_d = nc.declare_dram_parameter("mask", [BPC, S], I32, isOutput=False)
        web_d = nc.declare_dram_parameter("we_bc", [P, D], F32, isOutput=False)
        gdb_d = nc.declare_dram_parameter("gd_bc", [P, D], F32, isOutput=False)
        ge_d = nc.declare_dram_parameter("ge_row", [1, D], F32, isOutput=False)
        bg_d = nc.declare_dram_parameter("bgen", [1, 1], F32, isOutput=False)
        pw_d = nc.declare_dram_parameter("pw", [BPC, T, S], F32, isOutput=True)
        pg_d = nc.declare_dram_parameter("pgen", [BPC, T, 1], F32, isOutput=True)
        cx_d = nc.declare_dram_parameter("ctx", [BPC, T, D], F32, isOutput=True)

    with tile.TileContext(nc) as tc, ExitStack() as ectx:
        sb = ectx.enter_context(tc.tile_pool(name="sb", bufs=1))
        ps = ectx.enter_context(tc.tile_pool(name="ps", bufs=1, space="PSUM"))

        # ---- constants / parameter loads (bufs=1 tags: live forever) ----
        identity = sb.tile([P, P], F32, tag="identity", name="identity")
        make_identity(nc, identity)
        ones_col = sb.tile([P, 1], F32, tag="ones_col", name="ones_col")
        nc.vector.memset(ones_col, 1.0)
        ones_row = sb.tile([1, P], F32, tag="ones_row", name="ones_row")
        nc.vector.memset(ones_row, 1.0)
        zbias = sb.tile([P, 1], F32, tag="zbias", name="zbias")
        nc.vector.memset(zbias, 0.0)
        we_bc = sb.tile([P, D], F32, tag="we_bc", name="we_bc")
        nc.sync.dma_start(we_bc, web_d[:, :])
        gd_bc = sb.tile([P, D], F32, tag="gd_bc", name="gd_bc")
        nc.sync.dma_start(gd_bc, gdb_d[:, :])
        ge_row = sb.tile([1, D], F32, tag="ge_row", name="ge_row")
        nc.sync.dma_start(ge_row, ge_d[:, :])
        bgen_sb = sb.tile([1, 1], F32, tag="bgen", name="bgen_sb")
        nc.sync.dma_start(bgen_sb, bg_d[:, :])
        if bench_io:
            xt = sb.tile([1, 16], F32, tag="xt", name="xt")
            nc.sync.dma_start(xt, x_d[:, :])

        env = dict(enc_d=enc_d, dec_d=dec_d, mask_d=mask_d, pw_d=pw_d,
                   pg_d=pg_d, cx_d=cx_d, identity=identity, ones_col=ones_col,
                   ones_row=ones_row, zbias=zbias, we_bc=we_bc, gd_bc=gd_bc,
                   ge_row=ge_row, bgen_sb=bgen_sb)
        loop_cm = tc.For_i(0, loop_iters, 1) if loop_iters else nullcontext()
        with loop_cm:
            body(nc, tc, sb, ps, env)
        if bench_io:
            nc.sync.dma_start(y_d[:, :], xt)

    nc.compile()
    return nc


def body(nc, tc, sb, ps, env):
    enc_d, dec_d, mask_d = env["enc_d"], env["dec_d"], env["mask_d"]
    pw_d, pg_d, cx_d = env["pw_d"], env["pg_d"], env["cx_d"]
    identity, ones_col, zbias = env["identity"], env["ones_col"], env["zbias"]
    ones_row = env["ones_row"]
    we_bc, gd_bc, ge_row, bgen_sb = (env["we_bc"], env["gd_bc"],
                                     env["ge_row"], env["bgen_sb"])

    for b in range(BPC):
        # ---- mask first: load [16,128] i32, cast f32, PE-transpose ----
        mask_i = sb.tile([NS, P], I32, tag="mask_i", bufs=2, name=f"mask_i_{b}")
        nc.sync.dma_start(mask_i, mask_d[b].rearrange("(i f) -> i f", i=NS))
        mask_f = sb.tile([NS, P], F32, tag="mask_f", bufs=2, name=f"mask_f_{b}")
        nc.vector.tensor_copy(mask_f, mask_i)
        maskT_ps = ps.tile([P, NS], F32, tag="tp", bufs=2, name=f"maskT_ps_{b}")
        nc.tensor.transpose(maskT_ps, mask_f, identity[:NS, :NS])

        scores = sb.tile([P, NS], F32, tag="scores", bufs=2, name=f"scores_{b}")
        exp_c = sb.tile([P, NS], F32, tag="exp_c", bufs=2, name=f"exp_c_{b}")
        pm = sb.tile([P, NS], F32, tag="pm", bufs=2, name=f"pm_{b}")
        # context accumulator: column j holds ctx[128*j : 128*(j+1)] on the
        # partition dim.  enc slices go in as the (row-parallel) stationary
        # operand, the pm column streams as a single fp32 moving column.
        # start=True on any matmul zeroes the whole PSUM bank, which would
        # wipe sibling columns' already-accumulated chunk-0 results -- so
        # zero explicitly and accumulate with start=False throughout.
        if CTX_MODE == "stationary":
            ctx6_ps = ps.tile([P, ND], F32, tag="ctx", bufs=2,
                              name=f"ctx6_ps_{b}")
            nc.vector.memset(ctx6_ps, 0.0)
        else:
            ctxs_ps = ps.tile([1, D], F32, tag="ctx", bufs=1,
                              name=f"ctxs_ps_{b}")
            nc.vector.memset(ctxs_ps, 0.0)

        # ---- per-chunk pipeline: load -> mult -> reduce -> exp -> mask ->
        #      PE context accumulation.  Everything overlaps the DMA loads.
        enc_t = []
        for i in range(NS):
            et = sb.tile([P, D], F32, tag="enc", bufs=2 * NS, name=f"enc_{b}_{i}")
            nc.sync.dma_start(et, enc_d[b, P * i:P * (i + 1), :])
            enc_t.append(et)
            jt = sb.tile([P, D], F32, tag="junk", bufs=5, name=f"junk_{b}_{i}")
            nc.vector.tensor_tensor(out=jt, in0=et, in1=we_bc, op=MULT)
            jt2 = sb.tile([P, D], F32, tag="junk", bufs=5, name=f"junk2_{b}_{i}")
            nc.scalar.activation(jt2, jt, COPY, accum_out=scores[:, i:i + 1])
            nc.scalar.activation(exp_c[:, i:i + 1], scores[:, i:i + 1], EXP,
                                 bias=zbias)
            nc.vector.tensor_tensor(out=pm[:, i:i + 1], in0=exp_c[:, i:i + 1],
                                    in1=maskT_ps[:, i:i + 1], op=MULT)
            if CTX_MODE == "stationary":
                for j in range(ND):
                    nc.tensor.matmul(ctx6_ps[:, j:j + 1],
                                     et[:, P * j:P * (j + 1)], pm[:, i:i + 1],
                                     start=False, stop=(i == NS - 1),
                                     skip_group_check=True)
            else:
                for (c0, cn) in [(0, 512), (512, 256)]:
                    nc.tensor.matmul(ctxs_ps[:, c0:c0 + cn], pm[:, i:i + 1],
                                     et[:, c0:c0 + cn], start=False,
                                     stop=(i == NS - 1), skip_group_check=True)

        # ---- Z = sum(pm); rinv = 1/Z; broadcast via DMA ----
        rowsum = sb.tile([P, 1], F32, tag="rowsum", bufs=2, name=f"rowsum_{b}")
        nc.vector.tensor_reduce(out=rowsum, in_=pm, axis=AXF, op=ADD)
        z_ps = ps.tile([1, 1], F32, tag="zz", bufs=2, name=f"z_ps_{b}")
        nc.tensor.matmul(z_ps, rowsum, ones_col, start=True, stop=True)
        rinv = sb.tile([1, 1], F32, tag="rinv", bufs=2, name=f"rinv_{b}")
        nc.vector.reciprocal(rinv, z_ps)
        rb_ps = ps.tile([P, 1], F32, tag="zz", bufs=2, name=f"rb_ps_{b}")
        nc.tensor.matmul(rb_ps, ones_row, rinv, start=True, stop=True)
        rinv_bc = sb.tile([P, 1], F32, tag="rinv_bc", bufs=2, name=f"rinv_bc_{b}")
        nc.scalar.copy(rinv_bc, rb_ps)

        # ---- normalized p row: pm_n -> PE transpose -> [1, 2048] row ----
        pm_n = sb.tile([P, NS], F32, tag="pm_n", bufs=2, name=f"pm_n_{b}")
        nc.vector.tensor_scalar_mul(pm_n, pm, rinv_bc)
        pmT_ps = ps.tile([NS, P], F32, tag="tp", bufs=2, name=f"pmT_ps_{b}")
        nc.tensor.transpose(pmT_ps, pm_n, identity)
        pmT = sb.tile([NS, P], F32, tag="pmT", bufs=2, name=f"pmT_{b}")
        nc.scalar.copy(pmT, pmT_ps)
        p_row = sb.tile([1, S], F32, tag="p_row", bufs=2, name=f"p_row_{b}")
        nc.sync.dma_start(p_row.rearrange("a (i f) -> a i f", i=NS), pmT)

        # ---- pointer_weights: PE ones-matmul broadcast (reads the row once,
        #      fans out through the array), DVE copy to SBUF, out ----
        pw_t = sb.tile([P, S], F32, tag="pw_t", bufs=2, name=f"pw_t_{b}")
        for j in range(S // 512):
            bc_ps = ps.tile([P, 512], F32, tag="pwb", bufs=2,
                            name=f"pw_bc_{b}_{j}")
            nc.tensor.matmul(bc_ps, ones_row, p_row[0:1, 512 * j:512 * (j + 1)],
                             start=True, stop=True)
            nc.vector.tensor_copy(pw_t[:, 512 * j:512 * (j + 1)], bc_ps)
        for k in range(NT):
            nc.sync.dma_start(pw_d[b, P * k:P * (k + 1), :], pw_t)

        # ---- context row: normalize, transpose to a row, broadcast, out ----
        ctx_row = sb.tile([1, D], F32, tag="ctx_row", bufs=2, name=f"ctx_row_{b}")
        if CTX_MODE == "stationary":
            ctxn = sb.tile([P, ND], F32, tag="ctxn", bufs=2, name=f"ctxn_{b}")
            nc.vector.tensor_scalar_mul(ctxn, ctx6_ps, rinv_bc)
            ctxT_ps = ps.tile([ND, P], F32, tag="tp", bufs=2, name=f"ctxT_ps_{b}")
            nc.tensor.transpose(ctxT_ps, ctxn, identity)
            ctxT = sb.tile([ND, P], F32, tag="ctxT", bufs=2, name=f"ctxT_{b}")
            nc.scalar.copy(ctxT, ctxT_ps)
            nc.sync.dma_start(ctx_row.rearrange("a (j f) -> a j f", j=ND), ctxT)
        else:
            nc.vector.tensor_scalar_mul(ctx_row, ctxs_ps, rinv)
        ctx_t = sb.tile([P, D], F32, tag="ctx_t", bufs=2, name=f"ctx_t_{b}")
        for j, (c0, cn) in enumerate([(0, 512), (512, 256)]):
            cb_ps = ps.tile([P, 512], F32, tag="pwb", bufs=2,
                            name=f"cb_ps_{b}_{j}")
            nc.tensor.matmul(cb_ps[:, :cn], ones_row, ctx_row[0:1, c0:c0 + cn],
                             start=True, stop=True)
            nc.vector.tensor_copy(ctx_t[:, c0:c0 + cn], cb_ps[:, :cn])
        for k in range(NT):
            nc.sync.dma_start(cx_d[b, P * k:P * (k + 1), :], ctx_t)

        # ---- p_gen ----
        dsc = sb.tile([P, NT], F32, tag="dsc", bufs=2, name=f"dsc_{b}")
        for k in range(NT):
            dt_ = sb.tile([P, D], F32, tag="dec", bufs=2 * NT, name=f"dec_{b}_{k}")
            nc.sync.dma_start(dt_, dec_d[b, P * k:P * (k + 1), :])
            jd = sb.tile([P, D], F32, tag="junk", bufs=5, name=f"junkd_{b}_{k}")
            nc.vector.tensor_tensor(out=jd, in0=dt_, in1=gd_bc, op=MULT)
            jd2 = sb.tile([P, D], F32, tag="junk", bufs=5, name=f"junkd2_{b}_{k}")
            nc.scalar.activation(jd2, jd, COPY, accum_out=dsc[:, k:k + 1])
        # c2 = ctx_row . g_e + b_gen, broadcast to [128, 1] by DMA
        jr = sb.tile([1, D], F32, tag="junk1", bufs=2, name=f"jr_{b}")
        jr2 = sb.tile([1, D], F32, tag="junk1", bufs=2, name=f"jr2_{b}")
        c2 = sb.tile([1, 1], F32, tag="c2", bufs=2, name=f"c2_{b}")
        nc.vector.tensor_tensor(out=jr, in0=ctx_row, in1=ge_row, op=MULT)
        nc.scalar.activation(jr2, jr, COPY, accum_out=c2)
        c2b = sb.tile([1, 1], F32, tag="c2b", bufs=2, name=f"c2b_{b}")
        nc.vector.tensor_add(c2b, c2, bgen_sb)
        gb_ps = ps.tile([P, 1], F32, tag="zz", bufs=2, name=f"gb_ps_{b}")
        nc.tensor.matmul(gb_ps, ones_row, c2b, start=True, stop=True)
        gb = sb.tile([P, 1], F32, tag="gb", bufs=2, name=f"gb_{b}")
        nc.scalar.copy(gb, gb_ps)
        pgen_t = sb.tile([P, NT], F32, tag="pgen_t", bufs=2, name=f"pgen_t_{b}")
        nc.scalar.activation(pgen_t, dsc, SIGMOID, bias=gb)
        pgT_ps = ps.tile([NT, P], F32, tag="tp", bufs=2, name=f"pgT_ps_{b}")
        nc.tensor.transpose(pgT_ps, pgen_t, identity)
        pgT = sb.tile([NT, P], F32, tag="pgT", bufs=2, name=f"pgT_{b}")
        nc.scalar.copy(pgT, pgT_ps)
        nc.sync.dma_start(pg_d[b].rearrange("(k f) a -> k (f a)", k=NT), pgT)


def _run(inputs, trace=False, trace_cores=None, loop_iters=None):
    key = ("nc", loop_iters)
    if key not in _CACHE:
        _CACHE[key] = build_nc(loop_iters)
    nc = _CACHE[key]

    dec = np.ascontiguousarray(np.asarray(inputs["decoder_hidden"], dtype=np.float32))
    enc = np.ascontiguousarray(np.asarray(inputs["encoder_outputs"], dtype=np.float32))
    mask = np.ascontiguousarray(np.asarray(inputs["encoder_mask"], dtype=np.int32))
    w_ptr = np.asarray(inputs["w_ptr"], dtype=np.float32)
    w_gen = np.asarray(inputs["w_gen"], dtype=np.float32)
    b_gen = np.asarray(inputs["b_gen"], dtype=np.float32)
    we_bc = np.ascontiguousarray(np.broadcast_to(w_ptr[D:], (P, D)))
    gd_bc = np.ascontiguousarray(np.broadcast_to(w_gen[:D], (P, D)))
    ge_row = np.ascontiguousarray(w_gen[D:].reshape(1, D))
    bgen = np.ascontiguousarray(b_gen.reshape(1, 1))

    in_maps = []
    for c in range(NCORES):
        lo, hi = BPC * c, BPC * (c + 1)
        in_maps.append({
            "enc": np.ascontiguousarray(enc[lo:hi]),
            "dec": np.ascontiguousarray(dec[lo:hi]),
            "mask": np.ascontiguousarray(mask[lo:hi]),
            "we_bc": we_bc, "gd_bc": gd_bc, "ge_row": ge_row, "bgen": bgen,
        })

    res = run_bass_kernel_spmd(nc, in_maps, list(range(NCORES)),
                               trace=trace, trace_cores=trace_cores)
    pw = np.concatenate([r["pw"] for r in res.results], axis=0)
    pgen = np.concatenate([r["pgen"] for r in res.results], axis=0)
    ctx = np.concatenate([r["ctx"] for r in res.results], axis=0)
    return (pw, pgen, ctx), res


def kernel(decoder_hidden, encoder_outputs, encoder_mask, w_ptr, b_ptr,
           w_gen, b_gen):
    outs, _ = _run({
        "decoder_hidden": decoder_hidden,
        "encoder_outputs": encoder_outputs,
        "encoder_mask": encoder_mask,
        "w_ptr": w_ptr, "b_ptr": b_ptr, "w_gen": w_gen, "b_gen": b_gen,
    })
    return outs
